# revision 1
# baseline (speedup 1.0000x reference)
"""Trainium2 Bass kernel for nn_LogLinearAttention.

Math: the reference computes
    q = x@Wq.T+bq ; v = x@Wv.T+bv ; r = x@Wr.T+br
    scores = q @ v.T ; attn = softmax(scores, axis=1)   # over the QUERY axis
    emb[b,s,:] = sum_t attn[b,s,t] r[b,t,:] ; pooled = emb.sum(axis=1)
    out = sigmoid(pooled @ Wl.T + bl)

Because softmax normalizes over axis 1 and pooled sums over that same
axis, sum_s attn[s, t] == 1 for every t, so
    pooled[b] = sum_t r[b, t, :] = (sum_t x[b, t, :]) @ Wr.T + S*br
and the q/v projections and the S x S attention cancel exactly:
    out[b] = sigmoid( xsum[b] . (Wl@Wr) + S*(br . Wl) + bl )

The kernel therefore only needs a sequence-sum of x (the only large
input, 32MB total) plus tiny weight contractions. Data-parallel over
batch: core b handles x[b] (4MB), weights replicated.

Per-core device program (v4 — all fp32, HWDGE DMAs only):
  - x[b] arrives as 16 slice DMAs of [128,512] (256KB each), split
    across the sync and scalar HWDGE rings, all issued up-front
    (bufs=16) so arrival is continuous from ~8us.
  - acc[128,512] += slice on the vector engine as each slice lands; the
    DVE stream carries NOTHING but these adds until the tail, so a
    late-arriving weight DMA can never stall the chain.
  - All weights (Wr+Wl+br+bl) pack into ONE [128,2057] DMA (every DMA
    completion pays a ~3us straggler-engine lag, so count is minimized).
  - w_rep[128,512] = broadcast(Wl@Wr) on the TensorEngine via a
    free-dim-broadcast stationary operand; runs mid-stream.
  - The bias constant S*(br.Wl)+bl is computed entirely on GpSimd
    (tensor ops + XYZWC reduce), keeping DVE and PE clear.
  - tail: acc *= w_rep ; row-reduce ; 128->1 matmul with ones ;
    sigmoid (table prewarmed at kernel start) ; DMA the [1,1] out.
"""

import numpy as np

B, S, D = 8, 2048, 512
P = 128
NSL = 16  # x slice DMAs per core (256KB each)
JW = 4  # Wr/Wl/br rows per partition
ESH = D // B  # Wr column-shard width per core (AllGather reassembles)
WCOL = JW * ESH + 9  # packed weight columns: Wr shard | wl | br | bl pad
N_SYNC = 10  # slices on the sync ring; rest go on the scalar ring
# ring loads: sync 10 x 256KB = 2.5MB ; scalar = wp (1MB) + 6 x 256KB = 2.5MB

_CACHE = {}


def _build():
    import concourse.bacc as bacc
    import concourse.mybir as mybir
    import concourse.tile as tile

    f32 = mybir.dt.float32

    nc = bacc.Bacc(
        "TRN2",
        target_bir_lowering=False,
        debug=False,
        enable_asserts=False,
        num_devices=B,
    )
    x_d = nc.dram_tensor("x", [NSL, P, D], f32, kind="ExternalInput").ap()
    wp_d = nc.dram_tensor("wp", [P, WCOL], f32, kind="ExternalInput").ap()
    cc_in_d = nc.dram_tensor("cc_in", [P, ESH], f32).ap()
    cc_out_d = nc.dram_tensor("cc_out", [B, P, ESH], f32, addr_space="Shared").ap()
    out_d = nc.dram_tensor("out", [1, 1], f32, kind="ExternalOutput").ap()

    with tile.TileContext(nc) as tc:
        with (
            tc.tile_pool(name="xp", bufs=NSL) as xp,
            tc.tile_pool(name="sg", bufs=1) as sg,
            tc.tile_pool(name="ps", bufs=1, space="PSUM") as ps,
        ):
            ones = sg.tile([P, 1], f32, tag="ones")
            nc.vector.memset(ones, 1.0)
            # Prewarm the sigmoid activation table (1.3us) off the
            # critical path: a dummy [1,1] sigmoid right at the start.
            warm = sg.tile([1, 1], f32, tag="warm")
            nc.scalar.activation(
                warm, ones[0:1, 0:1], mybir.ActivationFunctionType.Sigmoid
            )

            # One DMA for every weight byte, first on the scalar ring.
            wp = sg.tile([P, WCOL], f32, tag="wp")
            nc.scalar.dma_start(wp, wp_d)
            wt = wp[:, : JW * ESH].rearrange("p (j e) -> p j e", j=JW)
            wlt = wp[:, JW * ESH : JW * ESH + JW]
            brt = wp[:, JW * ESH + JW : JW * ESH + 2 * JW]
            blt = wp[0:1, JW * ESH + 2 * JW : JW * ESH + 2 * JW + 1]

            # acc[128, D] accumulates the x stream on the vector engine.
            # One DMA + one add per 256KB slice; nothing else ever enters
            # the DVE stream before the tail.
            acc = sg.tile([P, D], f32, tag="acc")
            xts = []
            for n in range(NSL):
                xt = xp.tile([P, D], f32, tag="xt")
                eng = nc.sync if n < N_SYNC else nc.scalar
                eng.dma_start(xt, x_d[n])
                xts.append(xt)
                if n == 1:
                    nc.vector.tensor_add(out=acc, in0=xts[0], in1=xts[1])
                elif n > 1:
                    nc.vector.tensor_add(out=acc, in0=acc, in1=xt)

            # This core's 64-wide shard of w = Wl @ Wr, broadcast over
            # partitions: lhsT[k, p] = Wl[4k+j] for all p via a free-dim
            # broadcast of the [128,1] Wl column; rhs = Wr shard rows.
            wrep_ps = ps.tile([P, ESH], f32, tag="wrep")
            for j in range(JW):
                nc.tensor.matmul(
                    wrep_ps,
                    wlt[:, j : j + 1].to_broadcast([P, P]),
                    wt[:, j, :],
                    start=(j == 0),
                    stop=(j == JW - 1),
                )
            # AllGather the 8 shards into the full [128, D] broadcast w.
            # All hops ride GpSimd/SWDGE + the CC queue, far off the
            # critical path (done mid-stream).
            cc_in_sb = sg.tile([P, ESH], f32, tag="cc_in")
            nc.scalar.activation(
                cc_in_sb, wrep_ps, mybir.ActivationFunctionType.Copy
            )
            nc.gpsimd.dma_start(cc_in_d, cc_in_sb)
            nc.gpsimd.collective_compute(
                "AllGather",
                mybir.AluOpType.bypass,
                replica_groups=[list(range(B))],
                ins=[cc_in_d],
                outs=[cc_out_d],
            )
            w_rep3 = sg.tile([P, B, ESH], f32, tag="w_rep")
            nc.gpsimd.dma_start(w_rep3, cc_out_d.rearrange("r p e -> p r e"))
            w_rep = w_rep3.rearrange("p r e -> p (r e)")

            # constant term on GpSimd: t2 = S * dot(br, Wl) + bl
            prod2 = sg.tile([P, JW], f32, tag="prod2")
            nc.gpsimd.tensor_mul(out=prod2, in0=brt, in1=wlt)
            c_sb = sg.tile([1, 1], f32, tag="c_sb")
            nc.gpsimd.tensor_reduce(
                c_sb, prod2, axis=mybir.AxisListType.XYZWC, op=mybir.AluOpType.add
            )
            t2 = sg.tile([1, 1], f32, tag="t2")
            nc.gpsimd.tensor_scalar_mul(t2, c_sb, float(S))
            nc.gpsimd.tensor_add(out=t2, in0=t2, in1=blt)

            # tail: logit = sum_{p,d} acc*w_rep + t2 ; sigmoid.
            # (tensor_tensor_reduce would fuse these two DVE passes but
            # crashes the NEFF at execute time on this toolchain.)
            nc.vector.tensor_mul(out=acc, in0=acc, in1=w_rep)
            red = sg.tile([P, 1], f32, tag="red")
            nc.vector.reduce_sum(red, acc, axis=mybir.AxisListType.X)
            c2_ps = ps.tile([1, 1], f32, tag="c2")
            nc.tensor.matmul(c2_ps, red, ones, start=True, stop=True)
            fin = sg.tile([1, 1], f32, tag="fin")
            nc.scalar.activation(
                fin,
                c2_ps,
                mybir.ActivationFunctionType.Sigmoid,
                bias=t2,
                scale=1.0,
            )
            # out goes on the scalar ring: the sync ring is still
            # retiring the last x slices when fin is ready.
            nc.scalar.dma_start(out_d, fin)

    nc.compile()
    return nc


def _in_maps(inputs):
    x = np.ascontiguousarray(np.asarray(inputs["x"], dtype=np.float32))
    Wr = np.asarray(inputs["Wr"], dtype=np.float32)
    br = np.asarray(inputs["br"], dtype=np.float32)
    Wl = np.asarray(inputs["Wl"], dtype=np.float32)
    bl = np.asarray(inputs["bl"], dtype=np.float32)

    wr3 = Wr.reshape(P, JW, D)
    maps = []
    for b in range(B):
        wp = np.zeros((P, WCOL), dtype=np.float32)
        wp[:, : JW * ESH] = wr3[:, :, b * ESH : (b + 1) * ESH].reshape(P, JW * ESH)
        wp[:, JW * ESH : JW * ESH + JW] = Wl.reshape(P, JW)
        wp[:, JW * ESH + JW : JW * ESH + 2 * JW] = br.reshape(P, JW)
        wp[0, JW * ESH + 2 * JW] = bl[0]
        maps.append({"x": x[b].reshape(NSL, P, D), "wp": wp})
    return maps


def get_nc():
    if "nc" not in _CACHE:
        _CACHE["nc"] = _build()
    return _CACHE["nc"]


def kernel(**inputs) -> np.ndarray:
    from concourse.bass_utils import run_bass_kernel_spmd

    nc = get_nc()
    res = run_bass_kernel_spmd(nc, _in_maps(inputs), list(range(B)))
    out = np.stack([res.results[b]["out"].reshape(()) for b in range(B)])
    return out.reshape(B, 1).astype(np.float32)



# revision 2
# speedup vs baseline: 2.7823x; 2.7823x over previous
"""Trainium2 Bass kernel for nn_LogLinearAttention.

Math: the reference computes
    q = x@Wq.T+bq ; v = x@Wv.T+bv ; r = x@Wr.T+br
    scores = q @ v.T ; attn = softmax(scores, axis=1)   # over the QUERY axis
    emb[b,s,:] = sum_t attn[b,s,t] r[b,t,:] ; pooled = emb.sum(axis=1)
    out = sigmoid(pooled @ Wl.T + bl)

Because softmax normalizes over axis 1 and pooled sums over that same
axis, sum_s attn[s, t] == 1 for every t, so
    pooled[b] = sum_t r[b, t, :] = (sum_t x[b, t, :]) @ Wr.T + S*br
and the q/v projections and the S x S attention cancel exactly:
    out[b] = sigmoid( xsum[b] . w + c ),  w = (Wl@Wr)[0],
    c = S*(br . Wl[0]) + bl[0].

The kernel therefore only needs a sequence-sum of x (the only large
input, 32MB total) plus a tiny dot product. Data-parallel over batch:
core b handles x[b] (4MB), w/c replicated (precomputed host-side from
the D x D weights, like the packing any layout prep does).

Per-core device program (v5 — no collective, big DMAs):
  - x[b] arrives as NCH=4 chunk DMAs of [128, 2048] fp32 (1MB each) on
    the sync HWDGE ring, issued back-to-back up front.  1MB transfers
    run at ~341 GB/s vs ~210 GB/s for the 256KB slices v4 used.
  - acc4 = chunk0's tile; one wide DVE add per chunk (acc4 += chunk)
    as each lands, hidden under the next chunk's stream time.
  - w_rep[128,512] (w broadcast) + c pack into ONE [128,513] DMA on
    the scalar ring (263KB) — no PE weight matmuls, no AllGather (the
    v4 collective cost ~55us of start-delay + barrier + hop latency).
  - tail: fold acc4 2048->1024->512 ; acc *= w_rep ; row-reduce ;
    128->1 matmul with ones ; sigmoid+bias (table prewarmed) ; DMA the
    [1,1] out on the scalar ring.
"""

import numpy as np

B, S, D = 8, 2048, 512
P = 128
NCH = 4  # x chunk DMAs per core (1MB each)
CHC = 2048  # columns per chunk tile: [128, 2048] fp32 = 1MB = 512 rows of x
WCOL = D + 1  # packed: w broadcast | c

_CACHE = {}


def _build():
    import concourse.bacc as bacc
    import concourse.mybir as mybir
    import concourse.tile as tile

    f32 = mybir.dt.float32

    nc = bacc.Bacc(
        "TRN2",
        target_bir_lowering=False,
        debug=False,
        enable_asserts=False,
        num_devices=B,
    )
    x_d = nc.dram_tensor("x", [NCH, P, CHC], f32, kind="ExternalInput").ap()
    wp_d = nc.dram_tensor("wp", [P, WCOL], f32, kind="ExternalInput").ap()
    out_d = nc.dram_tensor("out", [1, 1], f32, kind="ExternalOutput").ap()

    with tile.TileContext(nc) as tc:
        with (
            tc.tile_pool(name="xp", bufs=NCH) as xp,
            tc.tile_pool(name="sg", bufs=1) as sg,
            tc.tile_pool(name="ps", bufs=1, space="PSUM") as ps,
        ):
            # x chunks first in the sync ring FIFO — nothing else rides it.
            xts = []
            for n in range(NCH):
                xt = xp.tile([P, CHC], f32, tag="xt")
                nc.sync.dma_start(xt, x_d[n])
                xts.append(xt)

            # Weights (w broadcast + c) in one small DMA, scalar ring.
            wp = sg.tile([P, WCOL], f32, tag="wp")
            nc.scalar.dma_start(wp, wp_d)
            w_rep = wp[:, :D]
            c_t = wp[0:1, D : D + 1]

            ones = sg.tile([P, 1], f32, tag="ones")
            nc.vector.memset(ones, 1.0)
            # Prewarm the sigmoid activation table (~1.3us) off the
            # critical path: a dummy [1,1] sigmoid right at the start.
            warm = sg.tile([1, 1], f32, tag="warm")
            nc.scalar.activation(
                warm, ones[0:1, 0:1], mybir.ActivationFunctionType.Sigmoid
            )

            # One wide add per chunk; acc4 is chunk 0's tile.
            acc4 = xts[0]
            for n in range(1, NCH):
                nc.vector.tensor_add(out=acc4, in0=acc4, in1=xts[n])
            # Fold 2048 -> 1024 -> 512 (in place; disjoint in1).
            nc.vector.tensor_add(
                out=acc4[:, 0:1024], in0=acc4[:, 0:1024], in1=acc4[:, 1024:2048]
            )
            nc.vector.tensor_add(
                out=acc4[:, 0:512], in0=acc4[:, 0:512], in1=acc4[:, 512:1024]
            )
            acc = acc4[:, 0:512]

            # tail: logit = sum_{p,d} acc*w_rep + c ; sigmoid.
            nc.vector.tensor_mul(out=acc, in0=acc, in1=w_rep)
            red = sg.tile([P, 1], f32, tag="red")
            nc.vector.reduce_sum(red, acc, axis=mybir.AxisListType.X)
            c2_ps = ps.tile([1, 1], f32, tag="c2")
            nc.tensor.matmul(c2_ps, red, ones, start=True, stop=True)
            fin = sg.tile([1, 1], f32, tag="fin")
            nc.scalar.activation(
                fin,
                c2_ps,
                mybir.ActivationFunctionType.Sigmoid,
                bias=c_t,
                scale=1.0,
            )
            nc.scalar.dma_start(out_d, fin)

    nc.compile()
    return nc


def _in_maps(inputs):
    x = np.ascontiguousarray(np.asarray(inputs["x"], dtype=np.float32))
    Wr = np.asarray(inputs["Wr"], dtype=np.float64)
    br = np.asarray(inputs["br"], dtype=np.float64)
    Wl = np.asarray(inputs["Wl"], dtype=np.float64)
    bl = np.asarray(inputs["bl"], dtype=np.float64)

    w = (Wl @ Wr).astype(np.float32)  # [1, D]
    c = float(S * (br @ Wl[0]) + bl[0])
    wp = np.empty((P, WCOL), dtype=np.float32)
    wp[:, :D] = w
    wp[:, D] = c

    return [{"x": x[b].reshape(NCH, P, CHC), "wp": wp} for b in range(B)]


def get_nc():
    if "nc" not in _CACHE:
        _CACHE["nc"] = _build()
    return _CACHE["nc"]


def kernel(**inputs) -> np.ndarray:
    from concourse.bass_utils import run_bass_kernel_spmd

    nc = get_nc()
    res = run_bass_kernel_spmd(nc, _in_maps(inputs), list(range(B)))
    out = np.stack([res.results[b]["out"].reshape(()) for b in range(B)])
    return out.reshape(B, 1).astype(np.float32)


# revision 4
# speedup vs baseline: 3.0421x; 1.0934x over previous
"""Trainium2 Bass kernel for nn_LogLinearAttention.

Math: the reference computes
    q = x@Wq.T+bq ; v = x@Wv.T+bv ; r = x@Wr.T+br
    scores = q @ v.T ; attn = softmax(scores, axis=1)   # over the QUERY axis
    emb[b,s,:] = sum_t attn[b,s,t] r[b,t,:] ; pooled = emb.sum(axis=1)
    out = sigmoid(pooled @ Wl.T + bl)

Because softmax normalizes over axis 1 and pooled sums over that same
axis, sum_s attn[s, t] == 1 for every t, so
    pooled[b] = sum_t r[b, t, :] = (sum_t x[b, t, :]) @ Wr.T + S*br
and the q/v projections and the S x S attention cancel exactly:
    out[b] = sigmoid( xsum[b] . w + c ),  w = (Wl@Wr)[0],
    c = S*(br . Wl[0]) + bl[0].

The kernel therefore only needs a sequence-sum of x (the only large
input, 32MB total) plus a tiny dot product. Data-parallel over batch:
core b handles x[b] (4MB), w/c replicated (precomputed host-side from
the D x D weights, like the packing any layout prep does).

Per-core device program (v6 — no collective, back-to-back DMA stream):
  - x[b] arrives as NCH=16 chunk DMAs of [128, 512] fp32 (256KB each)
    on the sync HWDGE ring, issued back-to-back up front.  Consecutive
    DMAs on one ring stream with no gap (v5's 4x1MB measured 361 GB/s
    across the whole window = the per-NC HBM limit), and the fine
    granularity lets each DVE add (0.69us) track the stream (0.73us
    per chunk), so only ONE add remains after the last byte lands —
    v5's 1MB chunks left a 2.3us add + 1.9us of folds in the tail.
  - acc = chunk0's tile; one [128,512] DVE add per chunk as it lands.
  - w_rep[128,512] (w broadcast) + c pack into ONE [128,513] DMA on
    the scalar ring (263KB) — no PE weight matmuls, no AllGather (the
    v4 collective cost ~55us of start-delay + barrier + hop latency).
  - tail: acc *= w_rep ; row-reduce ; 128->1 matmul with ones ;
    sigmoid+bias (table prewarmed) ; DMA the [1,1] out on the scalar
    ring.
"""

import numpy as np

B, S, D = 8, 2048, 512
P = 128
NCH = 16  # x chunk DMAs per core (256KB each)
CHC = 512  # columns per chunk tile: [128, 512] fp32 = 256KB = 128 rows of x
WCOL = D + 1  # packed: w broadcast | c

_CACHE = {}


def _build():
    import concourse.bacc as bacc
    import concourse.mybir as mybir
    import concourse.tile as tile

    f32 = mybir.dt.float32

    nc = bacc.Bacc(
        "TRN2",
        target_bir_lowering=False,
        debug=False,
        enable_asserts=False,
        num_devices=B,
    )
    x_d = nc.dram_tensor("x", [NCH, P, CHC], f32, kind="ExternalInput").ap()
    wp_d = nc.dram_tensor("wp", [P, WCOL], f32, kind="ExternalInput").ap()
    out_d = nc.dram_tensor("out", [1, 1], f32, kind="ExternalOutput").ap()

    with tile.TileContext(nc) as tc:
        with (
            tc.tile_pool(name="xp", bufs=NCH) as xp,
            tc.tile_pool(name="sg", bufs=1) as sg,
            tc.tile_pool(name="ps", bufs=1, space="PSUM") as ps,
        ):
            # x chunks first in the sync ring FIFO — nothing else rides it.
            xts = []
            for n in range(NCH):
                xt = xp.tile([P, CHC], f32, tag="xt")
                nc.sync.dma_start(xt, x_d[n])
                xts.append(xt)

            # Weights (w broadcast + c) in one small DMA, scalar ring.
            wp = sg.tile([P, WCOL], f32, tag="wp")
            nc.scalar.dma_start(wp, wp_d)
            w_rep = wp[:, :D]
            c_t = wp[0:1, D : D + 1]

            ones = sg.tile([P, 1], f32, tag="ones")
            nc.vector.memset(ones, 1.0)
            # Prewarm the sigmoid activation table (~1.3us) off the
            # critical path: a dummy [1,1] sigmoid right at the start.
            warm = sg.tile([1, 1], f32, tag="warm")
            nc.scalar.activation(
                warm, ones[0:1, 0:1], mybir.ActivationFunctionType.Sigmoid
            )

            # One add per chunk; acc is chunk 0's tile.
            acc = xts[0]
            for n in range(1, NCH):
                nc.vector.tensor_add(out=acc, in0=acc, in1=xts[n])

            # tail: logit = sum_{p,d} acc*w_rep + c ; sigmoid.
            nc.vector.tensor_mul(out=acc, in0=acc, in1=w_rep)
            red = sg.tile([P, 1], f32, tag="red")
            nc.vector.reduce_sum(red, acc, axis=mybir.AxisListType.X)
            c2_ps = ps.tile([1, 1], f32, tag="c2")
            nc.tensor.matmul(c2_ps, red, ones, start=True, stop=True)
            fin = sg.tile([1, 1], f32, tag="fin")
            nc.scalar.activation(
                fin,
                c2_ps,
                mybir.ActivationFunctionType.Sigmoid,
                bias=c_t,
                scale=1.0,
            )
            nc.scalar.dma_start(out_d, fin)

    nc.compile()
    return nc


def _in_maps(inputs):
    x = np.ascontiguousarray(np.asarray(inputs["x"], dtype=np.float32))
    Wr = np.asarray(inputs["Wr"], dtype=np.float64)
    br = np.asarray(inputs["br"], dtype=np.float64)
    Wl = np.asarray(inputs["Wl"], dtype=np.float64)
    bl = np.asarray(inputs["bl"], dtype=np.float64)

    w = (Wl @ Wr).astype(np.float32)  # [1, D]
    c = float(S * (br @ Wl[0]) + bl[0])
    wp = np.empty((P, WCOL), dtype=np.float32)
    wp[:, :D] = w
    wp[:, D] = c

    return [{"x": x[b].reshape(NCH, P, CHC), "wp": wp} for b in range(B)]


def get_nc():
    if "nc" not in _CACHE:
        _CACHE["nc"] = _build()
    return _CACHE["nc"]


def kernel(**inputs) -> np.ndarray:
    from concourse.bass_utils import run_bass_kernel_spmd

    nc = get_nc()
    res = run_bass_kernel_spmd(nc, _in_maps(inputs), list(range(B)))
    out = np.stack([res.results[b]["out"].reshape(()) for b in range(B)])
    return out.reshape(B, 1).astype(np.float32)
